# revision 11
# baseline (speedup 1.0000x reference)
"""Bimamba (bidirectional Mamba) block on 8 trn2 NeuronCores.

Sharding: tensor-parallel over d_inner (256 channels/core) for the mamba
body; LayerNorm is token-sharded (512 tokens/core) and the normalized
activations are AllGathered (two D-halves, overlapped with in_proj).
x_proj partial sums are AllReduced per direction in f16; the out_proj
contraction is resolved with a token-split AllToAll overlapped with
out_proj matmuls.
"""
import sys, os, json, time

sys.path.insert(0, '/opt/trn_rl_repo')

import numpy as np
import concourse.bass as bass
import concourse.mybir as mybir
import concourse.tile as tile
import bass_rust
from concourse.vector_clock import ScopedClock
from concourse import bass2jax
import jax

# ----------------------------------------------------------------- patches

def _patched_drain_and_barrier(self, tick_clock, wait_clock):
    nc = self.nc
    gc = tick_clock.global_clock
    vals = json.loads(repr(gc).replace("VectorClock(", "").rstrip(")"))
    procs = [i for i, v in enumerate(vals) if v > 0]
    for p in procs:
        sub = bass_rust.VectorClock()
        sub.require_at_least(p, vals[p])
        nop = nc.sync.nop(nofuse=True)
        wait_clock.add_sem_waits(nop.ins, ScopedClock({None: sub}))
    nc.sync.drain()
    nc.all_engine_barrier()
    assert self.sems is not None
    popped = nc._tile_sem_poison_stack.pop()
    assert popped is self._sem_poison
    nc.clear_and_free_semaphores(list(self.sems.allocated().values()))
    nc.all_engine_barrier()


tile.TileContext._drain_and_barrier = _patched_drain_and_barrier

_SPLIT_ENGINES = {"SP", "PE", "DVE", "Activation", "Pool"}
_wsplit_ctr = [0]


def _split_excess_waits(bir, max_waits=1):
    for f in bir.get("functions") or []:
        for blk in f.get("blocks") or []:
            insts = blk.get("instructions") or []
            out = []
            for inst in insts:
                si = inst.get("sync_info")
                waits = (si or {}).get("on_wait") or []
                eng = inst.get("engine")
                if len(waits) > max_waits and eng in _SPLIT_ENGINES:
                    keep, extra = waits[:max_waits], waits[max_waits:]
                    for i in range(0, len(extra), max_waits):
                        _wsplit_ctr[0] += 1
                        out.append({
                            "debug": inst.get("debug", 0),
                            "engine": eng,
                            "ins": [], "outs": [],
                            "name": f"WSPLIT-{_wsplit_ctr[0]}",
                            "opcode": "NoOp",
                            "sync_info": {"on_update": [],
                                          "on_wait": extra[i:i + max_waits]},
                        })
                    si["on_wait"] = keep
                out.append(inst)
            blk["instructions"] = out
    return bir


if not getattr(bass.Bass, "_ws_patched", False):
    _orig_to_json_bytes = bass.Bass.to_json_bytes

    def _patched_to_json_bytes(self):
        bir = json.loads(_orig_to_json_bytes(self))
        _split_excess_waits(bir)
        return json.dumps(bir).encode()

    bass.Bass.to_json_bytes = _patched_to_json_bytes
    bass.Bass._ws_patched = True

# ----------------------------------------------------------------- consts

B, D, L = 2, 1024, 2048
DIN, NST, DTR, KCV = 2048, 16, 64, 4
NC_ = 8
DL = DIN // NC_          # 256 channels per core
TOK = B * L              # 4096 tokens, b-major
TSL = TOK // NC_         # 512-token slice per core
EPS = 1e-5

f32 = mybir.dt.float32
f16 = mybir.dt.float16
AL = mybir.AluOpType
AF = mybir.ActivationFunctionType

SCAN_DT = f16            # dtype of scan operands (internal state is fp32)
NXP = DTR + 2 * NST      # 96


# ----------------------------------------------------------------- program

def build_program(reps=1):
    nc = bass.Bass(trn_type="TRN2", target_bir_lowering=False, num_devices=NC_)

    def din(name, shape, dt=f32):
        return nc.dram_tensor(name, list(shape), dt, kind="ExternalInput").ap()

    def dout(name, shape, dt=f32):
        return nc.dram_tensor(name, list(shape), dt, kind="ExternalOutput").ap()

    hss_in = din("hss", (D, TSL))           # per-core token slice
    ress_in = din("ress", (D, TSL))
    wx_in = din("wxT", (D, DL), f16)        # in_proj x-rows lhsT (gamma folded)
    wz_in = din("wzT", (D, DL), f16)
    bx_in = din("bx", (DL, 1))              # in_proj beta-fold biases
    bz_in = din("bz", (DL, 1))
    cvd_in = din("convdiag", (2, KCV, 2, 128, 128), f16)   # (dir,tap,dt,.,.)
    cb_in = din("convb", (2, DL, 1))
    xw_in = din("xwT", (2, DL, NXP), f16)   # (dir, k=dl, 96)
    dtw_in = din("dtwT", (2, DTR, DL), f16)
    dtb_in = din("dtb", (2, DL, 1))
    atab_in = din("atab", (2, DL, NST))
    dpd_in = din("dpdiag", (2, 2, 128, 128), f16)
    wop_in = din("wopT", (DIN, D), f16)
    opb_in = din("opb", (D, 1))
    i128_in = din("i128", (128, 128), f16)
    ones_in = din("ones", (128, 1), f16)

    r_out = dout("r_out", (D, TSL))         # per-core r token slice
    o_out = dout("o_out", (D, TSL))         # out token-slice (per core)
    ABL = set((os.environ.get("KERNEL_ABLATE") or "").split(","))

    with tile.TileContext(nc) as tc:
        with tc.tile_pool(name="wts", bufs=1) as wts, \
             tc.tile_pool(name="dram", bufs=1, space="DRAM") as dram:

            # ---- load small weights
            wx_sb = [wts.tile([128, DL], f16, tag=f"wx{k}", name=f"wx{k}") for k in range(8)]
            wz_sb = [wts.tile([128, DL], f16, tag=f"wz{k}", name=f"wz{k}") for k in range(8)]
            for k in range(8):
                nc.sync.dma_start(wx_sb[k][:], wx_in[k * 128:(k + 1) * 128, :])
                nc.sync.dma_start(wz_sb[k][:], wz_in[k * 128:(k + 1) * 128, :])
            bx_sb = [wts.tile([128, 1], f32, tag=f"bx{m}", name=f"bx{m}") for m in range(2)]
            bz_sb = [wts.tile([128, 1], f32, tag=f"bz{m}", name=f"bz{m}") for m in range(2)]
            for m in range(2):
                nc.sync.dma_start(bx_sb[m][:], bx_in[m * 128:(m + 1) * 128, :])
                nc.sync.dma_start(bz_sb[m][:], bz_in[m * 128:(m + 1) * 128, :])
            cvd_sb = {}
            for dr in range(2):
                for j in range(KCV):
                    for m in range(2):
                        t = wts.tile([128, 128], f16, tag=f"cv{dr}{j}{m}", name=f"cv{dr}{j}{m}")
                        nc.sync.dma_start(t[:], cvd_in[dr, j, m])
                        cvd_sb[dr, j, m] = t
            cb_sb = {}
            dtb_sb = {}
            at_sb = {}
            dpd_sb = {}
            for dr in range(2):
                for m in range(2):
                    t = wts.tile([128, 1], f32, tag=f"cb{dr}{m}", name=f"cb{dr}{m}")
                    nc.sync.dma_start(t[:], cb_in[dr, m * 128:(m + 1) * 128, :])
                    cb_sb[dr, m] = t
                    t = wts.tile([128, 1], f32, tag=f"db{dr}{m}", name=f"db{dr}{m}")
                    nc.sync.dma_start(t[:], dtb_in[dr, m * 128:(m + 1) * 128, :])
                    dtb_sb[dr, m] = t
                    t = wts.tile([128, NST], f32, tag=f"at{dr}{m}", name=f"at{dr}{m}")
                    nc.sync.dma_start(t[:], atab_in[dr, m * 128:(m + 1) * 128, :])
                    at_sb[dr, m] = t
                    t = wts.tile([128, 128], f16, tag=f"dp{dr}{m}", name=f"dp{dr}{m}")
                    nc.sync.dma_start(t[:], dpd_in[dr, m])
                    dpd_sb[dr, m] = t
            xw_sb = {}
            for dr in range(2):
                for m in range(2):
                    t = wts.tile([128, NXP], f16, tag=f"xw{dr}{m}", name=f"xw{dr}{m}")
                    nc.sync.dma_start(t[:], xw_in[dr, m * 128:(m + 1) * 128, :])
                    xw_sb[dr, m] = t
            dtw_sb = {}
            for dr in range(2):
                t = wts.tile([DTR, DL], f16, tag=f"dtw{dr}", name=f"dtw{dr}")
                nc.sync.dma_start(t[:], dtw_in[dr])
                dtw_sb[dr] = t
            i128_sb = wts.tile([128, 128], f16, tag="i128", name="i128")
            nc.sync.dma_start(i128_sb[:], i128_in)
            ones_sb = wts.tile([128, 1], f16, tag="ones", name="ones")
            nc.sync.dma_start(ones_sb[:], ones_in)
            opb_sb = [wts.tile([128, 1], f32, tag=f"opb{m}", name=f"opb{m}") for m in range(8)]
            for m in range(8):
                nc.sync.dma_start(opb_sb[m][:], opb_in[m * 128:(m + 1) * 128, :])

            for _rep in range(reps):
                # ---- explicit-lifetime activation pools (stack order!)
                cm_zs = tc.tile_pool(name="zsp", bufs=1)
                zsp = cm_zs.__enter__()
                zs16 = [zsp.tile([128, TOK], f16, tag=f"zs{m}", name=f"zs{m}") for m in range(2)]
                cm_u = tc.tile_pool(name="up", bufs=1)
                upool = cm_u.__enter__()
                u16 = {(dr, m): upool.tile([128, TOK], f16, tag=f"u{dr}{m}", name=f"u{dr}{m}")
                       for dr in range(2) for m in range(2)}
                cm_xp = tc.tile_pool(name="xpp", bufs=1)
                xpp = cm_xp.__enter__()
                xpad = {(m, b): xpp.tile([128, L + 6], f16, tag=f"xp{m}{b}", name=f"xp{m}{b}")
                        for m in range(2) for b in range(2)}
                for m in range(2):
                    for b in range(2):
                        nc.vector.memset(xpad[m, b][:, 0:3], 0.0)
                        nc.vector.memset(xpad[m, b][:, L + 3:L + 6], 0.0)

                # ======== Phase A: token-sharded LN on local 512 tokens ======
                ag_src = dram.tile([D, TSL], f16, tag="agsrc", name="agsrc")
                ag_dst = [dram.tile([NC_ * D // 2, TSL], f16, tag=f"agd{h}",
                                    name=f"agd{h}", addr_space="Shared")
                          for h in range(2)]
                cm_rhn = tc.tile_pool(name="rhn", bufs=1)
                rhn = cm_rhn.__enter__()
                with tc.tile_pool(name="lnw", bufs=1) as lnw, \
                     tc.tile_pool(name="lnps", bufs=1, space="PSUM") as lnps, \
                     tc.tile_pool(name="lnsm", bufs=1) as lnsm:
                    rs16 = [lnsm.tile([128, TSL], f16, tag=f"rs{k}", name=f"rs{k}")
                            for k in range(8)]
                    sst = lnps.tile([33, TSL], f32, tag="ss", name="ss")
                    ssum, ssq = sst[0:1, :], sst[32:33, :]
                    for k in range(8):
                        hs_t = lnw.tile([128, TSL], f32, tag="hs", name="hs_t", bufs=2)
                        re_t = lnw.tile([128, TSL], f32, tag="re", name="re_t", bufs=2)
                        nc.sync.dma_start(hs_t[:], hss_in[k * 128:(k + 1) * 128, :])
                        nc.sync.dma_start(re_t[:], ress_in[k * 128:(k + 1) * 128, :])
                        nc.vector.tensor_tensor(hs_t[:], hs_t[:], re_t[:], AL.add)
                        nc.sync.dma_start(r_out[k * 128:(k + 1) * 128, :], hs_t[:])
                        nc.vector.tensor_copy(rs16[k][:], hs_t[:])
                        sq_t = lnw.tile([128, TSL], f16, tag="sqt", name="sq_t", bufs=2)
                        nc.scalar.activation(sq_t[:], rs16[k][:], AF.Square)
                        nc.tensor.matmul(ssum, ones_sb[:], rs16[k][:],
                                         start=(k == 0), stop=(k == 7))
                        nc.tensor.matmul(ssq, ones_sb[:], sq_t[:],
                                         start=(k == 0), stop=(k == 7))
                    mu = lnsm.tile([1, TSL], f32, tag="mu", name="mu")
                    ex2 = lnsm.tile([1, TSL], f32, tag="ex2", name="ex2")
                    nc.vector.tensor_scalar_mul(mu[:], ssum, 1.0 / D)
                    nc.vector.tensor_scalar_mul(ex2[:], ssq, 1.0 / D)
                    tmp = lnsm.tile([1, TSL], f32, tag="tmp", name="tmp")
                    nc.vector.tensor_tensor(tmp[:], mu[:], mu[:], AL.mult)
                    nc.vector.tensor_tensor(ex2[:], ex2[:], tmp[:], AL.subtract)
                    nc.vector.tensor_scalar_add(ex2[:], ex2[:], float(EPS))
                    nc.scalar.activation(ex2[:], ex2[:], AF.Sqrt)
                    nc.vector.reciprocal(tmp[:], ex2[:])
                    r16_ = lnsm.tile([1, TSL], f16, tag="r16_", name="r16_")
                    m16_ = lnsm.tile([1, TSL], f16, tag="m16_", name="m16_")
                    nc.vector.tensor_copy(r16_[:], tmp[:])
                    nc.vector.tensor_copy(m16_[:], mu[:])
                    drow = dram.tile([2, TSL], f16, tag="stat", name="stat")
                    nc.sync.dma_start(drow[0:1, :], r16_[:])
                    nc.sync.dma_start(drow[1:2, :], m16_[:])
                    rb = lnsm.tile([128, TSL], f16, tag="rbc", name="rbc")
                    mb = lnsm.tile([128, TSL], f16, tag="mbc", name="mbc")
                    nc.sync.dma_start(rb[:], drow[0:1, :].broadcast_to((128, TSL)))
                    nc.sync.dma_start(mb[:], drow[1:2, :].broadcast_to((128, TSL)))
                    # hn_k = (r - mu) * rstd -> ag_src; AllGather per D-half
                    for k in range(8):
                        nc.vector.tensor_tensor(rs16[k][:], rs16[k][:], mb[:], AL.subtract)
                        nc.vector.tensor_tensor(rs16[k][:], rs16[k][:], rb[:], AL.mult)
                        nc.sync.dma_start(ag_src[k * 128:(k + 1) * 128, :], rs16[k][:])
                        if k == 3:
                            nc.gpsimd.collective_compute(
                                "AllGather", AL.bypass,
                                replica_groups=[list(range(NC_))],
                                ins=[ag_src[0:512, :].opt()], outs=[ag_dst[0].opt()])
                    nc.gpsimd.collective_compute(
                        "AllGather", AL.bypass, replica_groups=[list(range(NC_))],
                        ins=[ag_src[512:1024, :].opt()], outs=[ag_dst[1].opt()])

                # ======== Phase B: in_proj over gathered hn ==================
                with tc.tile_pool(name="bps", bufs=4, space="PSUM") as bps:
                    hn = [rhn.tile([128, TOK], f16, tag=f"hn{k}", name=f"hn{k}")
                          for k in range(8)]
                    for k in range(8):
                        h, ko = k // 4, (k % 4) * 128
                        for s in range(NC_):
                            nc.sync.dma_start(
                                hn[k][:, s * TSL:(s + 1) * TSL],
                                ag_dst[h][s * 512 + ko:s * 512 + ko + 128, :])
                    for m in range(4):      # 0,1 = x halves; 2,3 = z halves
                        for ch in range(8):
                            ps = bps.tile([128, 512], f32, tag="ps", name="ps")
                            for k in range(8):
                                w = wx_sb[k] if m < 2 else wz_sb[k]
                                lh = w[:, (m % 2) * 128:(m % 2) * 128 + 128]
                                nc.tensor.matmul(ps[:], lh,
                                                 hn[k][:, ch * 512:(ch + 1) * 512],
                                                 start=(k == 0), stop=(k == 7))
                            b, col = ch // 4, (ch % 4) * 512
                            if m < 2:
                                dst = xpad[m, b][:, 3 + col:3 + col + 512]
                                nc.scalar.activation(dst, ps[:], AF.Identity,
                                                     bias=bx_sb[m][:])
                            else:
                                dst = zs16[m - 2][:, ch * 512:ch * 512 + 512]
                                nc.scalar.activation(dst, ps[:], AF.Silu,
                                                     bias=bz_sb[m - 2][:])
                cm_rhn.__exit__(None, None, None)   # free hn

                # ======== Phase C+D: conv + x_proj + AR per direction ========
                ar_src = [dram.tile([NXP, TOK], f16, tag=f"ars{dr}", name=f"ars{dr}")
                          for dr in range(2)]
                ar_dst = [dram.tile([NXP, TOK], f16, tag=f"ard{dr}", name=f"ard{dr}",
                                    addr_space="Shared") for dr in range(2)]
                with tc.tile_pool(name="xrv", bufs=1) as xrv, \
                     tc.tile_pool(name="cwk", bufs=1) as cwk:
                    xrev = {}
                    for m in range(2):
                        for b in range(2):
                            t = xrv.tile([128, L + 6], f16, tag=f"xr{m}{b}", name=f"xr{m}{b}")
                            nc.vector.tensor_copy(t[:], xpad[m, b][:, L + 5::-1])
                            xrev[m, b] = t
                    with tc.tile_pool(name="cps", bufs=4, space="PSUM") as cps:
                        for dr in range(2):
                            # conv for this direction
                            for m in range(2):
                                for b in range(2):
                                    src_t = xpad[m, b] if dr == 0 else xrev[m, b]
                                    for c in range(4):
                                        ps = cps.tile([128, 512], f32, tag="ps", name="ps")
                                        for j in range(KCV):
                                            rhs = src_t[:, j + c * 512:j + c * 512 + 512]
                                            nc.tensor.matmul(ps[:], cvd_sb[dr, j, m], rhs,
                                                             start=(j == 0), stop=(j == KCV - 1))
                                        dst = u16[dr, m][:, b * L + c * 512:b * L + (c + 1) * 512]
                                        nc.scalar.activation(dst, ps[:], AF.Silu,
                                                             bias=cb_sb[dr, m][:])
                            # x_proj partials for this direction -> AR
                            for ch in range(8):
                                ps = cps.tile([NXP, 512], f32, tag="ps2", name="ps2")
                                for m in range(2):
                                    nc.tensor.matmul(ps[:], xw_sb[dr, m],
                                                     u16[dr, m][:, ch * 512:(ch + 1) * 512],
                                                     start=(m == 0), stop=(m == 1))
                                xc = cwk.tile([NXP, 512], f16, tag="xc", name="xc", bufs=3)
                                nc.scalar.activation(xc[:], ps[:], AF.Identity)
                                nc.sync.dma_start(
                                    ar_src[dr][:, ch * 512:(ch + 1) * 512], xc[:])
                            nc.gpsimd.collective_compute(
                                "AllReduce", AL.add, replica_groups=[list(range(NC_))],
                                ins=[ar_src[dr].opt()], outs=[ar_dst[dr].opt()])
                cm_xp.__exit__(None, None, None)    # free xpad
                cm_dt = tc.tile_pool(name="dtp", bufs=1)
                dtpool = cm_dt.__enter__()
                dt16 = {(dr, m): dtpool.tile([128, TOK], f16, tag=f"dt{dr}{m}", name=f"dt{dr}{m}")
                        for dr in range(2) for m in range(2)}
                dtu16 = {(dr, m): dtpool.tile([128, TOK], f16, tag=f"du{dr}{m}", name=f"du{dr}{m}")
                         for dr in range(2) for m in range(2)}
                y16 = [dtpool.tile([128, TOK], f16, tag=f"y{m}", name=f"y{m}") for m in range(2)]
                # dt chain per direction (dr=0 first so its scans start early)
                with tc.tile_pool(name="dps", bufs=4, space="PSUM") as dps, \
                     tc.tile_pool(name="dwk", bufs=1) as dwk:
                    for dr in range(2):
                        dtp16 = dwk.tile([DTR, TOK], f16, tag="dtp16", name="dtp16",
                                         bufs=2)
                        nc.sync.dma_start(dtp16[:], ar_dst[dr][0:DTR, :])
                        # dt = softplus(dtw @ dtpart + dtb) via Exp then Ln(x+1)
                        for m in range(2):
                            for ch in range(8):
                                ps = dps.tile([128, 512], f32, tag="psd", name="psd")
                                nc.tensor.matmul(ps[:],
                                                 dtw_sb[dr][:, m * 128:(m + 1) * 128],
                                                 dtp16[:, ch * 512:(ch + 1) * 512],
                                                 start=True, stop=True)
                                et = dwk.tile([128, 512], f32, tag="et", name="et", bufs=3)
                                nc.scalar.activation(et[:], ps[:], AF.Exp,
                                                     bias=dtb_sb[dr, m][:])
                                nc.scalar.activation(
                                    dt16[dr, m][:, ch * 512:(ch + 1) * 512], et[:],
                                    AF.Ln, bias=1.0)
                        for m in range(2):
                            nc.vector.tensor_tensor(dtu16[dr, m][:], dt16[dr, m][:],
                                                    u16[dr, m][:], AL.mult)

                # ================= Phase E: selective scan ===================
                for dr in range(2):
                    for b in range(2):
                        bsl = slice(b * L, (b + 1) * L)
                        with tc.tile_pool(name=f"eps{dr}{b}", bufs=1, space="PSUM") as eps, \
                             tc.tile_pool(name=f"esw{dr}{b}", bufs=2) as esw:
                            py = {(m, c): eps.tile([128, 512], f32, tag=f"py{m}{c}", name=f"py{m}{c}")
                                  for m in range(2) for c in range(4)}
                            for n in range(NST):
                                if "nobc" not in ABL:
                                    bt = esw.tile([128, L], f16, tag="bt", name="bt")
                                    nc.sync.dma_start(
                                        bt[:], ar_dst[dr][DTR + n:DTR + n + 1,
                                                          bsl].broadcast_to((128, L)))
                                    ct = esw.tile([128, L], f16, tag="ct", name="ct")
                                    nc.sync.dma_start(
                                        ct[:], ar_dst[dr][DTR + NST + n:DTR + NST + n + 1,
                                                          bsl].broadcast_to((128, L)))
                                for m in range(2):
                                    h16 = esw.tile([128, L], SCAN_DT, tag="h16", name="h16")
                                    if "noscan" not in ABL:
                                        a16 = esw.tile([128, L], SCAN_DT, tag="a16", name="a16")
                                        nc.scalar.activation(a16[:], dt16[dr, m][:, bsl],
                                                             AF.Exp,
                                                             scale=at_sb[dr, m][:, n:n + 1])
                                        xs = esw.tile([128, L], SCAN_DT, tag="xs", name="xs")
                                        nc.vector.tensor_tensor(xs[:], dtu16[dr, m][:, bsl],
                                                                bt[:], AL.mult)
                                        nc.vector.tensor_tensor_scan(h16[:], a16[:], xs[:],
                                                                     0.0, AL.mult, AL.add)
                                        nc.vector.tensor_tensor(h16[:], h16[:], ct[:], AL.mult)
                                    if "nonsum" not in ABL:
                                        for c in range(4):
                                            nc.tensor.matmul(py[m, c][:], i128_sb[:],
                                                             h16[:, c * 512:(c + 1) * 512],
                                                             start=(n == 0), stop=False)
                                    elif n == 0:
                                        for c in range(4):
                                            nc.tensor.matmul(py[m, c][:], i128_sb[:],
                                                             h16[:, c * 512:(c + 1) * 512],
                                                             start=True, stop=False)
                            for m in range(2):
                                for c in range(4):
                                    nc.tensor.matmul(
                                        py[m, c][:], dpd_sb[dr, m],
                                        u16[dr, m][:, b * L + c * 512:b * L + (c + 1) * 512],
                                        start=False, stop=True)
                            for m in range(2):
                                for c in range(4):
                                    csl = slice(b * L + c * 512, b * L + (c + 1) * 512)
                                    if dr == 0:
                                        nc.vector.tensor_tensor(y16[m][:, csl], py[m, c][:],
                                                                zs16[m][:, csl], AL.mult)
                                    else:
                                        gt = esw.tile([128, 512], f16, tag="gt", name="gt")
                                        rc = 3 - c
                                        rev = py[m, rc][:, 511::-1]
                                        nc.vector.tensor_tensor(gt[:], rev,
                                                                zs16[m][:, csl], AL.mult)
                                        nc.vector.tensor_tensor(y16[m][:, csl],
                                                                y16[m][:, csl], gt[:], AL.add)

                # ============ Phase F+G: A2A halves + out_proj ==============
                HT = TSL // 2
                y_src = [dram.tile([DIN, HT], f16, tag=f"ysrc{h}", name=f"ysrc{h}")
                         for h in range(2)]
                y_dst = [dram.tile([DIN, HT], f16, tag=f"ydst{h}", name=f"ydst{h}")
                         for h in range(2)]
                for h in range(2):
                    for j in range(NC_):
                        for m in range(2):
                            nc.sync.dma_start(
                                y_src[h][j * DL + m * 128:j * DL + (m + 1) * 128, :],
                                y16[m][:, j * TSL + h * HT:j * TSL + (h + 1) * HT])
                    nc.gpsimd.collective_compute(
                        "AllToAll", AL.bypass, replica_groups=[list(range(NC_))],
                        ins=[y_src[h].opt()], outs=[y_dst[h].opt()])
                cm_dt.__exit__(None, None, None)
                cm_u.__exit__(None, None, None)
                cm_zs.__exit__(None, None, None)

                with tc.tile_pool(name="gps", bufs=4, space="PSUM") as gps, \
                     tc.tile_pool(name="gwk", bufs=3) as gwk, \
                     tc.tile_pool(name="gya", bufs=1) as gya:
                    wop_sb = [gya.tile([128, D], f16, tag=f"wo{k}", name=f"wo{k}")
                              for k in range(16)]
                    for k in range(16):
                        nc.sync.dma_start(wop_sb[k][:], wop_in[k * 128:(k + 1) * 128, :])
                    for h in range(2):
                        yall = [gya.tile([128, HT], f16, tag=f"ya{h}{k}", name=f"ya{h}{k}")
                                for k in range(16)]
                        for k in range(16):
                            nc.sync.dma_start(yall[k][:], y_dst[h][k * 128:(k + 1) * 128, :])
                        for mt in range(8):
                            ps = gps.tile([128, HT], f32, tag="ps", name="ps")
                            for k in range(16):
                                nc.tensor.matmul(ps[:], wop_sb[k][:, mt * 128:(mt + 1) * 128],
                                                 yall[k][:], start=(k == 0), stop=(k == 15))
                            o32 = gwk.tile([128, HT], f32, tag="o32", name="o32")
                            nc.scalar.activation(o32[:], ps[:], AF.Identity,
                                                 bias=opb_sb[mt][:])
                            nc.sync.dma_start(
                                o_out[mt * 128:(mt + 1) * 128, h * HT:(h + 1) * HT], o32[:])
    return nc


# ----------------------------------------------------------------- host

def _host_prep(inputs):
    """Build per-core input dicts from the full-model inputs."""
    gam = np.asarray(inputs["gamma"], np.float32)
    bet = np.asarray(inputs["beta"], np.float32)
    wip = np.asarray(inputs["in_proj_w"], np.float32)     # (2*DIN, D)
    wop = np.asarray(inputs["out_proj_w"], np.float32)    # (D, DIN)
    opb = np.asarray(inputs["out_proj_b"], np.float32)
    hs = np.asarray(inputs["hidden_states"], np.float32)
    res = np.asarray(inputs["residual"], np.float32)

    conv_w = [np.asarray(inputs["conv_w"], np.float32),
              np.asarray(inputs["conv_w_b"], np.float32)]
    conv_b = [np.asarray(inputs["conv_b"], np.float32),
              np.asarray(inputs["conv_b_b"], np.float32)]
    xw = [np.asarray(inputs["xproj_w"], np.float32),
          np.asarray(inputs["xproj_w_b"], np.float32)]
    dtw = [np.asarray(inputs["dtproj_w"], np.float32),
           np.asarray(inputs["dtproj_w_b"], np.float32)]
    dtb = [np.asarray(inputs["dtproj_b"], np.float32),
           np.asarray(inputs["dtproj_b_b"], np.float32)]
    alog = [np.asarray(inputs["A_log"], np.float32),
            np.asarray(inputs["A_b_log"], np.float32)]
    dp = [np.asarray(inputs["Dp"], np.float32),
          np.asarray(inputs["Dp_b"], np.float32)]

    wip_g = wip * gam[None, :]           # fold gamma
    bias_full = wip @ bet                # fold beta  (2*DIN,)

    i128 = np.eye(128, dtype=np.float16)
    ones = np.ones((128, 1), np.float16)

    # token-major flattening of hs/res: (B, D, L) -> (D, B*L)
    hs_f = hs.transpose(1, 0, 2).reshape(D, TOK)
    res_f = res.transpose(1, 0, 2).reshape(D, TOK)

    in_maps = []
    for i in range(NC_):
        ds = slice(i * DL, (i + 1) * DL)
        wxT = wip_g[ds, :].T.astype(np.float16)               # (D, DL)
        wzT = wip_g[DIN + i * DL:DIN + (i + 1) * DL, :].T.astype(np.float16)
        bx = bias_full[ds].reshape(DL, 1).astype(np.float32)
        bz = bias_full[DIN + i * DL:DIN + (i + 1) * DL].reshape(DL, 1).astype(np.float32)
        cvd = np.zeros((2, KCV, 2, 128, 128), np.float16)
        cb = np.zeros((2, DL, 1), np.float32)
        xwT = np.zeros((2, DL, DTR + 2 * NST), np.float16)
        dtwT = np.zeros((2, DTR, DL), np.float16)
        dtbv = np.zeros((2, DL, 1), np.float32)
        atab = np.zeros((2, DL, NST), np.float32)
        dpd = np.zeros((2, 2, 128, 128), np.float16)
        for dr in range(2):
            w = conv_w[dr][ds, 0, :]                          # (DL, KCV)
            for j in range(KCV):
                for m in range(2):
                    cvd[dr, j, m] = np.diag(w[m * 128:(m + 1) * 128, j]).astype(np.float16)
            cb[dr] = conv_b[dr][ds].reshape(DL, 1)
            xwT[dr] = xw[dr][:, ds].T.astype(np.float16)      # (DL, 96)
            dtwT[dr] = dtw[dr][ds, :].T.astype(np.float16)    # (DTR, DL)
            dtbv[dr] = dtb[dr][ds].reshape(DL, 1)
            atab[dr] = -np.exp(alog[dr][ds, :])
            for m in range(2):
                dpd[dr, m] = np.diag(dp[dr][ds][m * 128:(m + 1) * 128]).astype(np.float16)
        in_maps.append({
            "hss": np.ascontiguousarray(hs_f[:, i * TSL:(i + 1) * TSL]),
            "ress": np.ascontiguousarray(res_f[:, i * TSL:(i + 1) * TSL]),
            "wxT": wxT, "wzT": wzT, "bx": bx, "bz": bz,
            "convdiag": cvd, "convb": cb,
            "xwT": xwT, "dtwT": dtwT, "dtb": dtbv, "atab": atab,
            "dpdiag": dpd,
            "wopT": wop.T.astype(np.float16),                 # (DIN, D)
            "opb": opb.reshape(D, 1).astype(np.float32),
            "i128": i128, "ones": ones,
        })
    return in_maps


class _Exec:
    """Compile once; run via PJRT shard_map on 8 cores."""

    def __init__(self, nc, n_cores):
        from jax.sharding import Mesh, PartitionSpec
        from jax.experimental.shard_map import shard_map
        bass2jax.install_neuronx_cc_hook()
        self.nc = nc
        self.n = n_cores
        partition_name = nc.partition_id_tensor.name if nc.partition_id_tensor else None
        in_names, out_names, out_avals, zero_outs = [], [], [], []
        for alloc in nc.m.functions[0].allocations:
            if not isinstance(alloc, mybir.MemoryLocationSet):
                continue
            name = alloc.memorylocations[0].name
            if alloc.kind == "ExternalInput":
                if name != partition_name:
                    in_names.append(name)
            elif alloc.kind == "ExternalOutput":
                shape = tuple(alloc.tensor_shape)
                npdt = mybir.dt.np(alloc.dtype)
                out_names.append(name)
                out_avals.append(jax.core.ShapedArray(shape, npdt))
                zero_outs.append(np.zeros(shape, npdt))
        self.in_names, self.out_names = in_names, out_names
        self.out_avals, self.zero_outs = out_avals, zero_outs
        all_in = list(in_names) + list(out_names)
        if partition_name is not None:
            all_in.append(partition_name)

        def _body(*args):
            operands = list(args)
            if partition_name is not None:
                operands.append(bass2jax.partition_id_tensor())
            outs = bass2jax._bass_exec_p.bind(
                *operands,
                out_avals=tuple(out_avals),
                in_names=tuple(all_in),
                out_names=tuple(out_names),
                lowering_input_output_aliases=(),
                sim_require_finite=True,
                sim_require_nnan=True,
                nc=nc,
            )
            return tuple(outs)

        devices = jax.devices()[:n_cores]
        self.mesh = Mesh(np.asarray(devices), ("core",))
        np_ = len(in_names) + len(out_names)
        self.fn = jax.jit(
            shard_map(_body, mesh=self.mesh,
                      in_specs=(PartitionSpec("core"),) * np_,
                      out_specs=(PartitionSpec("core"),) * len(out_names),
                      check_rep=False),
            keep_unused=True)

    def prep(self, in_maps):
        from jax.sharding import NamedSharding, PartitionSpec
        n = self.n
        cat = [np.concatenate([np.asarray(in_maps[c][k]) for c in range(n)], axis=0)
               for k in self.in_names]
        cat += [np.zeros((n * z.shape[0], *z.shape[1:]), z.dtype)
                for z in self.zero_outs]
        sh = NamedSharding(self.mesh, PartitionSpec("core"))
        return [jax.device_put(a, sh) for a in cat]

    def run(self, args):
        outs = self.fn(*args)
        jax.block_until_ready(outs)
        return outs

    def results(self, outs):
        n = self.n
        return [
            {name: np.asarray(outs[i]).reshape(n, *self.out_avals[i].shape)[c]
             for i, name in enumerate(self.out_names)}
            for c in range(n)
        ]


_EXEC = None


def _get_exec():
    global _EXEC
    if _EXEC is None:
        _EXEC = _Exec(build_program(), NC_)
    return _EXEC


def kernel(**inputs):
    e = _get_exec()
    in_maps = _host_prep(inputs)
    res = e.results(e.run(e.prep(in_maps)))
    out = np.zeros((B, D, L), np.float32)
    r = np.zeros((B, D, L), np.float32)
    for i in range(NC_):
        b = i // 4
        l0 = (i % 4) * TSL
        out[b][:, l0:l0 + TSL] = res[i]["o_out"]
        r[b][:, l0:l0 + TSL] = res[i]["r_out"]
    return out, r


# revision 43
# speedup vs baseline: 2.3157x; 2.3157x over previous
"""Bimamba (bidirectional Mamba) block on 8 trn2 NeuronCores.

Sharding: tensor-parallel over d_inner (256 channels/core) for the mamba
body; LayerNorm is token-sharded (512 tokens/core) and the normalized
activations are AllGathered (two D-halves, overlapped with in_proj).
x_proj partial sums are AllReduced per direction in f16; the out_proj
contraction is resolved with a token-split AllToAll overlapped with
out_proj matmuls.
"""
import sys, os, json, time

sys.path.insert(0, '/opt/trn_rl_repo')

import numpy as np
import concourse.bass as bass
import concourse.mybir as mybir
import concourse.tile as tile
import bass_rust
from concourse.vector_clock import ScopedClock
from concourse import bass2jax
import jax

# ----------------------------------------------------------------- patches

def _patched_drain_and_barrier(self, tick_clock, wait_clock):
    nc = self.nc
    gc = tick_clock.global_clock
    vals = json.loads(repr(gc).replace("VectorClock(", "").rstrip(")"))
    procs = [i for i, v in enumerate(vals) if v > 0]
    for p in procs:
        sub = bass_rust.VectorClock()
        sub.require_at_least(p, vals[p])
        nop = nc.sync.nop(nofuse=True)
        wait_clock.add_sem_waits(nop.ins, ScopedClock({None: sub}))
    nc.sync.drain()
    nc.all_engine_barrier()
    assert self.sems is not None
    popped = nc._tile_sem_poison_stack.pop()
    assert popped is self._sem_poison
    nc.clear_and_free_semaphores(list(self.sems.allocated().values()))
    nc.all_engine_barrier()


tile.TileContext._drain_and_barrier = _patched_drain_and_barrier

_SPLIT_ENGINES = {"SP", "PE", "DVE", "Activation", "Pool"}
_wsplit_ctr = [0]


def _split_excess_waits(bir, max_waits=1):
    for f in bir.get("functions") or []:
        for blk in f.get("blocks") or []:
            insts = blk.get("instructions") or []
            out = []
            for inst in insts:
                si = inst.get("sync_info")
                waits = (si or {}).get("on_wait") or []
                eng = inst.get("engine")
                if len(waits) > max_waits and eng in _SPLIT_ENGINES:
                    keep, extra = waits[:max_waits], waits[max_waits:]
                    for i in range(0, len(extra), max_waits):
                        _wsplit_ctr[0] += 1
                        out.append({
                            "debug": inst.get("debug", 0),
                            "engine": eng,
                            "ins": [], "outs": [],
                            "name": f"WSPLIT-{_wsplit_ctr[0]}",
                            "opcode": "NoOp",
                            "sync_info": {"on_update": [],
                                          "on_wait": extra[i:i + max_waits]},
                        })
                    si["on_wait"] = keep
                out.append(inst)
            blk["instructions"] = out
    return bir


if not getattr(bass.Bass, "_ws_patched", False):
    _orig_to_json_bytes = bass.Bass.to_json_bytes

    def _patched_to_json_bytes(self):
        bir = json.loads(_orig_to_json_bytes(self))
        _split_excess_waits(bir)
        return json.dumps(bir).encode()

    bass.Bass.to_json_bytes = _patched_to_json_bytes
    bass.Bass._ws_patched = True

# ----------------------------------------------------------------- consts

B, D, L = 2, 1024, 2048
DIN, NST, DTR, KCV = 2048, 16, 64, 4
NC_ = 8
DL = DIN // NC_          # 256 channels per core
TOK = B * L              # 4096 tokens, b-major
TSL = TOK // NC_         # 512-token slice per core
EPS = 1e-5

f32 = mybir.dt.float32
f16 = mybir.dt.float16
f8 = mybir.dt.float8e4
F8NP = mybir.dt.np(f8)
AL = mybir.AluOpType
AF = mybir.ActivationFunctionType

SCAN_DT = f16            # dtype of scan operands (internal state is fp32)
NXP = DTR + 2 * NST      # 96


# ----------------------------------------------------------------- program

def build_program(reps=1):
    nc = bass.Bass(trn_type="TRN2", target_bir_lowering=False, num_devices=NC_)

    def din(name, shape, dt=f32):
        return nc.dram_tensor(name, list(shape), dt, kind="ExternalInput").ap()

    def dout(name, shape, dt=f32):
        return nc.dram_tensor(name, list(shape), dt, kind="ExternalOutput").ap()

    hss_in = din("hss", (D, TSL))           # per-core token slice
    ress_in = din("ress", (D, TSL))
    wx_in = din("wxT", (D, DL), f16)        # in_proj x-rows lhsT (gamma folded)
    wz_in = din("wzT", (D, DL), f16)
    bx_in = din("bx", (DL, 1))              # in_proj beta-fold biases
    bz_in = din("bz", (DL, 1))
    cvd_in = din("convdiag", (2, KCV, 2, 128, 128), f16)   # (dir,tap,dt,.,.)
    cb_in = din("convb", (2, DL, 1))
    xw_in = din("xwT", (2, DL, NXP), f16)   # (dir, k=dl, 96)
    dtw_in = din("dtwT", (2, DTR, DL), f16)
    dtb_in = din("dtb", (2, DL, 1))
    atab_in = din("atab", (2, DL, NST))
    dpd_in = din("dpdiag", (2, 2, 128, 128), f16)
    wop_in = din("wopT", (DIN, D), f16)
    opb_in = din("opb", (D, 1))
    i128_in = din("i128", (128, 128), f16)
    ones_in = din("ones", (128, 1), f16)

    r_out = dout("r_out", (D, TSL))         # per-core r token slice
    o_out = dout("o_out", (D, TSL))         # out token-slice (per core)
    ABL = set((os.environ.get("KERNEL_ABLATE") or "").split(","))

    with tile.TileContext(nc) as tc:
        with tc.tile_pool(name="wts", bufs=1) as wts, \
             tc.tile_pool(name="dram", bufs=1, space="DRAM") as dram:

            # ---- load small weights
            wx_sb = [wts.tile([128, DL], f16, tag=f"wx{k}", name=f"wx{k}") for k in range(8)]
            wz_sb = [wts.tile([128, DL], f16, tag=f"wz{k}", name=f"wz{k}") for k in range(8)]
            for k in range(8):
                nc.sync.dma_start(wx_sb[k][:], wx_in[k * 128:(k + 1) * 128, :])
                nc.sync.dma_start(wz_sb[k][:], wz_in[k * 128:(k + 1) * 128, :])
            bx_sb = [wts.tile([128, 1], f32, tag=f"bx{m}", name=f"bx{m}") for m in range(2)]
            bz_sb = [wts.tile([128, 1], f32, tag=f"bz{m}", name=f"bz{m}") for m in range(2)]
            for m in range(2):
                nc.sync.dma_start(bx_sb[m][:], bx_in[m * 128:(m + 1) * 128, :])
                nc.sync.dma_start(bz_sb[m][:], bz_in[m * 128:(m + 1) * 128, :])
            cvd_sb = {}
            for dr in range(2):
                for j in range(KCV):
                    for m in range(2):
                        t = wts.tile([128, 128], f16, tag=f"cv{dr}{j}{m}", name=f"cv{dr}{j}{m}")
                        nc.sync.dma_start(t[:], cvd_in[dr, j, m])
                        cvd_sb[dr, j, m] = t
            cb_sb = {}
            dtb_sb = {}
            at_sb = {}
            dpd_sb = {}
            for dr in range(2):
                for m in range(2):
                    t = wts.tile([128, 1], f32, tag=f"cb{dr}{m}", name=f"cb{dr}{m}")
                    nc.sync.dma_start(t[:], cb_in[dr, m * 128:(m + 1) * 128, :])
                    cb_sb[dr, m] = t
                    t = wts.tile([128, 1], f32, tag=f"db{dr}{m}", name=f"db{dr}{m}")
                    nc.sync.dma_start(t[:], dtb_in[dr, m * 128:(m + 1) * 128, :])
                    dtb_sb[dr, m] = t
                    t = wts.tile([128, NST], f32, tag=f"at{dr}{m}", name=f"at{dr}{m}")
                    nc.sync.dma_start(t[:], atab_in[dr, m * 128:(m + 1) * 128, :])
                    at_sb[dr, m] = t
                    t = wts.tile([128, 128], f16, tag=f"dp{dr}{m}", name=f"dp{dr}{m}")
                    nc.sync.dma_start(t[:], dpd_in[dr, m])
                    dpd_sb[dr, m] = t
            xw_sb = {}
            for dr in range(2):
                for m in range(2):
                    t = wts.tile([128, NXP], f16, tag=f"xw{dr}{m}", name=f"xw{dr}{m}")
                    nc.sync.dma_start(t[:], xw_in[dr, m * 128:(m + 1) * 128, :])
                    xw_sb[dr, m] = t
            dtw_sb = {}
            for dr in range(2):
                t = wts.tile([DTR, DL], f16, tag=f"dtw{dr}", name=f"dtw{dr}")
                nc.sync.dma_start(t[:], dtw_in[dr])
                dtw_sb[dr] = t
            i128_sb = wts.tile([128, 128], f16, tag="i128", name="i128")
            nc.sync.dma_start(i128_sb[:], i128_in)
            ones_sb = wts.tile([128, 1], f16, tag="ones", name="ones")
            nc.sync.dma_start(ones_sb[:], ones_in)
            opb_sb = [wts.tile([128, 1], f32, tag=f"opb{m}", name=f"opb{m}") for m in range(8)]
            for m in range(8):
                nc.sync.dma_start(opb_sb[m][:], opb_in[m * 128:(m + 1) * 128, :])

            for _rep in range(reps):
                # ---- explicit-lifetime activation pools (stack order!)
                cm_zs = tc.tile_pool(name="zsp", bufs=1)
                zsp = cm_zs.__enter__()
                zs16 = [zsp.tile([128, TOK], f16, tag=f"zs{m}", name=f"zs{m}") for m in range(2)]
                cm_u = tc.tile_pool(name="up", bufs=1)
                upool = cm_u.__enter__()
                u16 = {(dr, m): upool.tile([128, TOK], f16, tag=f"u{dr}{m}", name=f"u{dr}{m}")
                       for dr in range(2) for m in range(2)}
                cm_xp = tc.tile_pool(name="xpp", bufs=1)
                xpp = cm_xp.__enter__()
                xpad = {(m, b): xpp.tile([128, L + 6], f16, tag=f"xp{m}{b}", name=f"xp{m}{b}")
                        for m in range(2) for b in range(2)}
                for m in range(2):
                    for b in range(2):
                        nc.vector.memset(xpad[m, b][:, 0:3], 0.0)
                        nc.vector.memset(xpad[m, b][:, L + 3:L + 6], 0.0)

                # ======== Phase A: token-sharded LN on local 512 tokens ======
                ag_src = dram.tile([D, TSL], f16, tag="agsrc", name="agsrc")
                ag_dst = [dram.tile([NC_ * D // 2, TSL], f16, tag=f"agd{h}",
                                    name=f"agd{h}", addr_space="Shared")
                          for h in range(2)]
                cm_rhn = tc.tile_pool(name="rhn", bufs=1)
                rhn = cm_rhn.__enter__()
                with tc.tile_pool(name="lnw", bufs=1) as lnw, \
                     tc.tile_pool(name="lnps", bufs=1, space="PSUM") as lnps, \
                     tc.tile_pool(name="lnsm", bufs=1) as lnsm:
                    rs16 = [lnsm.tile([128, TSL], f16, tag=f"rs{k}", name=f"rs{k}")
                            for k in range(8)]
                    sst = lnps.tile([33, TSL], f32, tag="ss", name="ss")
                    ssum, ssq = sst[0:1, :], sst[32:33, :]
                    for k in range(8):
                        hs_t = lnw.tile([128, TSL], f32, tag="hs", name="hs_t", bufs=2)
                        re_t = lnw.tile([128, TSL], f32, tag="re", name="re_t", bufs=2)
                        nc.sync.dma_start(hs_t[:], hss_in[k * 128:(k + 1) * 128, :])
                        nc.sync.dma_start(re_t[:], ress_in[k * 128:(k + 1) * 128, :])
                        nc.vector.tensor_tensor(hs_t[:], hs_t[:], re_t[:], AL.add)
                        nc.sync.dma_start(r_out[k * 128:(k + 1) * 128, :], hs_t[:])
                        nc.vector.tensor_copy(rs16[k][:], hs_t[:])
                        sq_t = lnw.tile([128, TSL], f16, tag="sqt", name="sq_t", bufs=2)
                        nc.scalar.activation(sq_t[:], rs16[k][:], AF.Square)
                        nc.tensor.matmul(ssum, ones_sb[:], rs16[k][:],
                                         start=(k == 0), stop=(k == 7))
                        nc.tensor.matmul(ssq, ones_sb[:], sq_t[:],
                                         start=(k == 0), stop=(k == 7))
                    mu = lnsm.tile([1, TSL], f32, tag="mu", name="mu")
                    ex2 = lnsm.tile([1, TSL], f32, tag="ex2", name="ex2")
                    nc.vector.tensor_scalar_mul(mu[:], ssum, 1.0 / D)
                    nc.vector.tensor_scalar_mul(ex2[:], ssq, 1.0 / D)
                    tmp = lnsm.tile([1, TSL], f32, tag="tmp", name="tmp")
                    nc.vector.tensor_tensor(tmp[:], mu[:], mu[:], AL.mult)
                    nc.vector.tensor_tensor(ex2[:], ex2[:], tmp[:], AL.subtract)
                    nc.vector.tensor_scalar_add(ex2[:], ex2[:], float(EPS))
                    nc.scalar.activation(ex2[:], ex2[:], AF.Sqrt)
                    nc.vector.reciprocal(tmp[:], ex2[:])
                    r16_ = lnsm.tile([1, TSL], f16, tag="r16_", name="r16_")
                    m16_ = lnsm.tile([1, TSL], f16, tag="m16_", name="m16_")
                    nc.vector.tensor_copy(r16_[:], tmp[:])
                    nc.vector.tensor_copy(m16_[:], mu[:])
                    drow = dram.tile([2, TSL], f16, tag="stat", name="stat")
                    nc.sync.dma_start(drow[0:1, :], r16_[:])
                    nc.sync.dma_start(drow[1:2, :], m16_[:])
                    rb = lnsm.tile([128, TSL], f16, tag="rbc", name="rbc")
                    mb = lnsm.tile([128, TSL], f16, tag="mbc", name="mbc")
                    nc.sync.dma_start(rb[:], drow[0:1, :].broadcast_to((128, TSL)))
                    nc.sync.dma_start(mb[:], drow[1:2, :].broadcast_to((128, TSL)))
                    # hn_k = (r - mu) * rstd -> ag_src; AllGather per D-half
                    for k in range(8):
                        nc.vector.tensor_tensor(rs16[k][:], rs16[k][:], mb[:], AL.subtract)
                        nc.vector.tensor_tensor(rs16[k][:], rs16[k][:], rb[:], AL.mult)
                        nc.sync.dma_start(ag_src[k * 128:(k + 1) * 128, :], rs16[k][:])
                        if k == 3:
                            nc.gpsimd.collective_compute(
                                "AllGather", AL.bypass,
                                replica_groups=[list(range(NC_))],
                                ins=[ag_src[0:512, :].opt()], outs=[ag_dst[0].opt()])
                    nc.gpsimd.collective_compute(
                        "AllGather", AL.bypass, replica_groups=[list(range(NC_))],
                        ins=[ag_src[512:1024, :].opt()], outs=[ag_dst[1].opt()])

                # ======== Phase B: in_proj over gathered hn ==================
                with tc.tile_pool(name="bps", bufs=4, space="PSUM") as bps:
                    hn = [rhn.tile([128, TOK], f16, tag=f"hn{k}", name=f"hn{k}")
                          for k in range(8)]
                    for k in range(8):
                        h, ko = k // 4, (k % 4) * 128
                        for s in range(NC_):
                            nc.sync.dma_start(
                                hn[k][:, s * TSL:(s + 1) * TSL],
                                ag_dst[h][s * 512 + ko:s * 512 + ko + 128, :])
                    for m in range(4):      # 0,1 = x halves; 2,3 = z halves
                        for ch in range(8):
                            ps = bps.tile([128, 512], f32, tag="ps", name="ps")
                            for k in range(8):
                                w = wx_sb[k] if m < 2 else wz_sb[k]
                                lh = w[:, (m % 2) * 128:(m % 2) * 128 + 128]
                                nc.tensor.matmul(ps[:], lh,
                                                 hn[k][:, ch * 512:(ch + 1) * 512],
                                                 start=(k == 0), stop=(k == 7))
                            b, col = ch // 4, (ch % 4) * 512
                            if m < 2:
                                dst = xpad[m, b][:, 3 + col:3 + col + 512]
                                nc.scalar.activation(dst, ps[:], AF.Identity,
                                                     bias=bx_sb[m][:])
                            else:
                                dst = zs16[m - 2][:, ch * 512:ch * 512 + 512]
                                nc.scalar.activation(dst, ps[:], AF.Silu,
                                                     bias=bz_sb[m - 2][:])
                cm_rhn.__exit__(None, None, None)   # free hn

                # ======== Phase C+D: conv + x_proj + AR per direction ========
                ar_src = [dram.tile([NXP, TOK], f16, tag=f"ars{dr}", name=f"ars{dr}")
                          for dr in range(2)]
                ar_dst = [dram.tile([NXP, TOK], f16, tag=f"ard{dr}", name=f"ard{dr}",
                                    addr_space="Shared") for dr in range(2)]
                with tc.tile_pool(name="xrv", bufs=1) as xrv, \
                     tc.tile_pool(name="cwk", bufs=1) as cwk:
                    xrev = {}
                    for m in range(2):
                        for b in range(2):
                            t = xrv.tile([128, L + 6], f16, tag=f"xr{m}{b}", name=f"xr{m}{b}")
                            nc.vector.tensor_copy(t[:], xpad[m, b][:, L + 5::-1])
                            xrev[m, b] = t
                    with tc.tile_pool(name="cps", bufs=4, space="PSUM") as cps:
                        for dr in range(2):
                            # conv for this direction
                            for m in range(2):
                                for b in range(2):
                                    src_t = xpad[m, b] if dr == 0 else xrev[m, b]
                                    for c in range(4):
                                        ps = cps.tile([128, 512], f32, tag="ps", name="ps")
                                        for j in range(KCV):
                                            rhs = src_t[:, j + c * 512:j + c * 512 + 512]
                                            nc.tensor.matmul(ps[:], cvd_sb[dr, j, m], rhs,
                                                             start=(j == 0), stop=(j == KCV - 1))
                                        dst = u16[dr, m][:, b * L + c * 512:b * L + (c + 1) * 512]
                                        nc.scalar.activation(dst, ps[:], AF.Silu,
                                                             bias=cb_sb[dr, m][:])
                            # x_proj partials for this direction -> AR
                            for ch in range(8):
                                ps = cps.tile([NXP, 512], f32, tag="ps2", name="ps2")
                                for m in range(2):
                                    nc.tensor.matmul(ps[:], xw_sb[dr, m],
                                                     u16[dr, m][:, ch * 512:(ch + 1) * 512],
                                                     start=(m == 0), stop=(m == 1))
                                xc = cwk.tile([NXP, 512], f16, tag="xc", name="xc", bufs=3)
                                nc.scalar.activation(xc[:], ps[:], AF.Identity)
                                nc.sync.dma_start(
                                    ar_src[dr][:, ch * 512:(ch + 1) * 512], xc[:])
                            nc.gpsimd.collective_compute(
                                "AllReduce", AL.add, replica_groups=[list(range(NC_))],
                                ins=[ar_src[dr].opt()], outs=[ar_dst[dr].opt()])
                cm_xp.__exit__(None, None, None)    # free xpad
                cm_dt = tc.tile_pool(name="dtp", bufs=1)
                dtpool = cm_dt.__enter__()
                dt16 = {(dr, m): dtpool.tile([128, TOK], f16, tag=f"dt{dr}{m}", name=f"dt{dr}{m}")
                        for dr in range(2) for m in range(2)}
                dtu16 = {(dr, m): dtpool.tile([128, TOK], f16, tag=f"du{dr}{m}", name=f"du{dr}{m}")
                         for dr in range(2) for m in range(2)}
                y16 = [dtpool.tile([128, TOK], f16, tag=f"y{m}", name=f"y{m}") for m in range(2)]

                def emit_dt(dr):
                    # dt = softplus(dtw @ dtpart + dtb) via Exp then Ln(x+1)
                    with tc.tile_pool(name=f"dps{dr}", bufs=2, space="PSUM") as dps, \
                         tc.tile_pool(name=f"dwk{dr}", bufs=1) as dwk:
                        dtp16 = dwk.tile([DTR, TOK], f16, tag="dtp16", name="dtp16")
                        nc.sync.dma_start(dtp16[:], ar_dst[dr][0:DTR, :])
                        for m in range(2):
                            for ch in range(8):
                                ps = dps.tile([128, 512], f32, tag="psd", name="psd")
                                nc.tensor.matmul(ps[:],
                                                 dtw_sb[dr][:, m * 128:(m + 1) * 128],
                                                 dtp16[:, ch * 512:(ch + 1) * 512],
                                                 start=True, stop=True)
                                et = dwk.tile([128, 512], f32, tag="et", name="et", bufs=3)
                                nc.scalar.activation(et[:], ps[:], AF.Exp,
                                                     bias=dtb_sb[dr, m][:])
                                nc.scalar.activation(
                                    dt16[dr, m][:, ch * 512:(ch + 1) * 512], et[:],
                                    AF.Ln, bias=1.0)
                        for m in range(2):
                            nc.vector.tensor_tensor(dtu16[dr, m][:], dt16[dr, m][:],
                                                    u16[dr, m][:], AL.mult)

                def emit_scan(dr, b):
                    bsl = slice(b * L, (b + 1) * L)
                    with tc.tile_pool(name=f"eps{dr}{b}", bufs=1, space="PSUM") as eps, \
                         tc.tile_pool(name=f"esw{dr}{b}", bufs=2) as esw:
                        py = {(m, c): eps.tile([128, 512], f32, tag=f"py{m}{c}", name=f"py{m}{c}")
                              for m in range(2) for c in range(4)}
                        for n in range(NST):
                            if "nobc" not in ABL:
                                bt = esw.tile([128, L], f16, tag="bt", name="bt")
                                nc.sync.dma_start(
                                    bt[:], ar_dst[dr][DTR + n:DTR + n + 1,
                                                      bsl].broadcast_to((128, L)))
                                ct = esw.tile([128, L], f16, tag="ct", name="ct")
                                nc.sync.dma_start(
                                    ct[:], ar_dst[dr][DTR + NST + n:DTR + NST + n + 1,
                                                      bsl].broadcast_to((128, L)))
                            for m in range(2):
                                h16 = esw.tile([128, L], SCAN_DT, tag="h16", name="h16")
                                if "noscan" not in ABL:
                                    a16 = esw.tile([128, L], SCAN_DT, tag="a16", name="a16")
                                    nc.scalar.activation(a16[:], dt16[dr, m][:, bsl],
                                                         AF.Exp,
                                                         scale=at_sb[dr, m][:, n:n + 1])
                                    xs = esw.tile([128, L], SCAN_DT, tag="xs", name="xs")
                                    nc.vector.tensor_tensor(xs[:], dtu16[dr, m][:, bsl],
                                                            bt[:], AL.mult)
                                    nc.vector.tensor_tensor_scan(h16[:], a16[:], xs[:],
                                                                 0.0, AL.mult, AL.add)
                                    nc.vector.tensor_tensor(h16[:], h16[:], ct[:], AL.mult)
                                if "nonsum" not in ABL:
                                    for c in range(4):
                                        nc.tensor.matmul(py[m, c][:], i128_sb[:],
                                                         h16[:, c * 512:(c + 1) * 512],
                                                         start=(n == 0), stop=False)
                                elif n == 0:
                                    for c in range(4):
                                        nc.tensor.matmul(py[m, c][:], i128_sb[:],
                                                         h16[:, c * 512:(c + 1) * 512],
                                                         start=True, stop=False)
                        for m in range(2):
                            for c in range(4):
                                nc.tensor.matmul(
                                    py[m, c][:], dpd_sb[dr, m],
                                    u16[dr, m][:, b * L + c * 512:b * L + (c + 1) * 512],
                                    start=False, stop=True)
                        for m in range(2):
                            for c in range(4):
                                csl = slice(b * L + c * 512, b * L + (c + 1) * 512)
                                if dr == 0:
                                    nc.vector.tensor_tensor(y16[m][:, csl], py[m, c][:],
                                                            zs16[m][:, csl], AL.mult)
                                else:
                                    gt = esw.tile([128, 512], f16, tag="gt", name="gt")
                                    rc = 3 - c
                                    rev = py[m, rc][:, 511::-1]
                                    nc.vector.tensor_tensor(gt[:], rev,
                                                            zs16[m][:, csl], AL.mult)
                                    nc.vector.tensor_tensor(y16[m][:, csl],
                                                            y16[m][:, csl], gt[:], AL.add)

                # ====== Phase D2+E: dt chains interleaved with scans =========
                emit_dt(0)
                emit_scan(0, 0)
                emit_dt(1)          # PSUM free between scan blocks; AR1 landed
                emit_scan(0, 1)
                emit_scan(1, 0)
                emit_scan(1, 1)

                # ============ Phase F+G: A2A halves + out_proj ==============
                HT = TSL // 2
                y_src = [dram.tile([DIN, HT], f16, tag=f"ysrc{h}", name=f"ysrc{h}")
                         for h in range(2)]
                y_dst = [dram.tile([DIN, HT], f16, tag=f"ydst{h}", name=f"ydst{h}")
                         for h in range(2)]
                for h in range(2):
                    for j in range(NC_):
                        for m in range(2):
                            nc.sync.dma_start(
                                y_src[h][j * DL + m * 128:j * DL + (m + 1) * 128, :],
                                y16[m][:, j * TSL + h * HT:j * TSL + (h + 1) * HT])
                    nc.gpsimd.collective_compute(
                        "AllToAll", AL.bypass, replica_groups=[list(range(NC_))],
                        ins=[y_src[h].opt()], outs=[y_dst[h].opt()])
                cm_dt.__exit__(None, None, None)
                cm_u.__exit__(None, None, None)
                cm_zs.__exit__(None, None, None)

                with tc.tile_pool(name="gps", bufs=4, space="PSUM") as gps, \
                     tc.tile_pool(name="gwk", bufs=3) as gwk, \
                     tc.tile_pool(name="gya", bufs=1) as gya:
                    wop_sb = [gya.tile([128, D], f16, tag=f"wo{k}", name=f"wo{k}")
                              for k in range(16)]
                    for k in range(16):
                        nc.sync.dma_start(wop_sb[k][:], wop_in[k * 128:(k + 1) * 128, :])
                    for h in range(2):
                        yall = [gya.tile([128, HT], f16, tag=f"ya{h}{k}", name=f"ya{h}{k}")
                                for k in range(16)]
                        for k in range(16):
                            nc.sync.dma_start(yall[k][:], y_dst[h][k * 128:(k + 1) * 128, :])
                        for mt in range(8):
                            ps = gps.tile([128, HT], f32, tag="ps", name="ps")
                            for k in range(16):
                                nc.tensor.matmul(ps[:], wop_sb[k][:, mt * 128:(mt + 1) * 128],
                                                 yall[k][:], start=(k == 0), stop=(k == 15))
                            o32 = gwk.tile([128, HT], f32, tag="o32", name="o32")
                            nc.scalar.activation(o32[:], ps[:], AF.Identity,
                                                 bias=opb_sb[mt][:])
                            nc.sync.dma_start(
                                o_out[mt * 128:(mt + 1) * 128, h * HT:(h + 1) * HT], o32[:])
    return nc


# ----------------------------------------------------------------- host

def _host_prep(inputs):
    """Build per-core input dicts from the full-model inputs."""
    gam = np.asarray(inputs["gamma"], np.float32)
    bet = np.asarray(inputs["beta"], np.float32)
    wip = np.asarray(inputs["in_proj_w"], np.float32)     # (2*DIN, D)
    wop = np.asarray(inputs["out_proj_w"], np.float32)    # (D, DIN)
    opb = np.asarray(inputs["out_proj_b"], np.float32)
    hs = np.asarray(inputs["hidden_states"], np.float32)
    res = np.asarray(inputs["residual"], np.float32)

    conv_w = [np.asarray(inputs["conv_w"], np.float32),
              np.asarray(inputs["conv_w_b"], np.float32)]
    conv_b = [np.asarray(inputs["conv_b"], np.float32),
              np.asarray(inputs["conv_b_b"], np.float32)]
    xw = [np.asarray(inputs["xproj_w"], np.float32),
          np.asarray(inputs["xproj_w_b"], np.float32)]
    dtw = [np.asarray(inputs["dtproj_w"], np.float32),
           np.asarray(inputs["dtproj_w_b"], np.float32)]
    dtb = [np.asarray(inputs["dtproj_b"], np.float32),
           np.asarray(inputs["dtproj_b_b"], np.float32)]
    alog = [np.asarray(inputs["A_log"], np.float32),
            np.asarray(inputs["A_b_log"], np.float32)]
    dp = [np.asarray(inputs["Dp"], np.float32),
          np.asarray(inputs["Dp_b"], np.float32)]

    wip_g = wip * gam[None, :]           # fold gamma
    bias_full = wip @ bet                # fold beta  (2*DIN,)

    i128 = np.eye(128, dtype=np.float16)
    ones = np.ones((128, 1), np.float16)

    # token-major flattening of hs/res: (B, D, L) -> (D, B*L)
    hs_f = hs.transpose(1, 0, 2).reshape(D, TOK)
    res_f = res.transpose(1, 0, 2).reshape(D, TOK)

    in_maps = []
    for i in range(NC_):
        ds = slice(i * DL, (i + 1) * DL)
        wxT = wip_g[ds, :].T.astype(np.float16)               # (D, DL)
        wzT = wip_g[DIN + i * DL:DIN + (i + 1) * DL, :].T.astype(np.float16)
        bx = bias_full[ds].reshape(DL, 1).astype(np.float32)
        bz = bias_full[DIN + i * DL:DIN + (i + 1) * DL].reshape(DL, 1).astype(np.float32)
        cvd = np.zeros((2, KCV, 2, 128, 128), np.float16)
        cb = np.zeros((2, DL, 1), np.float32)
        xwT = np.zeros((2, DL, DTR + 2 * NST), np.float16)
        dtwT = np.zeros((2, DTR, DL), np.float16)
        dtbv = np.zeros((2, DL, 1), np.float32)
        atab = np.zeros((2, DL, NST), np.float32)
        dpd = np.zeros((2, 2, 128, 128), np.float16)
        for dr in range(2):
            w = conv_w[dr][ds, 0, :]                          # (DL, KCV)
            for j in range(KCV):
                for m in range(2):
                    cvd[dr, j, m] = np.diag(w[m * 128:(m + 1) * 128, j]).astype(np.float16)
            cb[dr] = conv_b[dr][ds].reshape(DL, 1)
            xwT[dr] = xw[dr][:, ds].T.astype(np.float16)      # (DL, 96)
            dtwT[dr] = dtw[dr][ds, :].T.astype(np.float16)    # (DTR, DL)
            dtbv[dr] = dtb[dr][ds].reshape(DL, 1)
            atab[dr] = -np.exp(alog[dr][ds, :])
            for m in range(2):
                dpd[dr, m] = np.diag(dp[dr][ds][m * 128:(m + 1) * 128]).astype(np.float16)
        in_maps.append({
            "hss": np.ascontiguousarray(hs_f[:, i * TSL:(i + 1) * TSL]),
            "ress": np.ascontiguousarray(res_f[:, i * TSL:(i + 1) * TSL]),
            "wxT": wxT, "wzT": wzT, "bx": bx, "bz": bz,
            "convdiag": cvd, "convb": cb,
            "xwT": xwT, "dtwT": dtwT, "dtb": dtbv, "atab": atab,
            "dpdiag": dpd,
            "wopT": wop.T.astype(np.float16),                 # (DIN, D)
            "opb": opb.reshape(D, 1).astype(np.float32),
            "i128": i128, "ones": ones,
        })
    return in_maps


class _Exec:
    """Compile once; run via PJRT shard_map on 8 cores."""

    def __init__(self, nc, n_cores):
        from jax.sharding import Mesh, PartitionSpec
        from jax.experimental.shard_map import shard_map
        bass2jax.install_neuronx_cc_hook()
        self.nc = nc
        self.n = n_cores
        partition_name = nc.partition_id_tensor.name if nc.partition_id_tensor else None
        in_names, out_names, out_avals, zero_outs = [], [], [], []
        for alloc in nc.m.functions[0].allocations:
            if not isinstance(alloc, mybir.MemoryLocationSet):
                continue
            name = alloc.memorylocations[0].name
            if alloc.kind == "ExternalInput":
                if name != partition_name:
                    in_names.append(name)
            elif alloc.kind == "ExternalOutput":
                shape = tuple(alloc.tensor_shape)
                npdt = mybir.dt.np(alloc.dtype)
                out_names.append(name)
                out_avals.append(jax.core.ShapedArray(shape, npdt))
                zero_outs.append(np.zeros(shape, npdt))
        self.in_names, self.out_names = in_names, out_names
        self.out_avals, self.zero_outs = out_avals, zero_outs
        all_in = list(in_names) + list(out_names)
        if partition_name is not None:
            all_in.append(partition_name)

        def _body(*args):
            operands = list(args)
            if partition_name is not None:
                operands.append(bass2jax.partition_id_tensor())
            outs = bass2jax._bass_exec_p.bind(
                *operands,
                out_avals=tuple(out_avals),
                in_names=tuple(all_in),
                out_names=tuple(out_names),
                lowering_input_output_aliases=(),
                sim_require_finite=True,
                sim_require_nnan=True,
                nc=nc,
            )
            return tuple(outs)

        devices = jax.devices()[:n_cores]
        self.mesh = Mesh(np.asarray(devices), ("core",))
        np_ = len(in_names) + len(out_names)
        self.fn = jax.jit(
            shard_map(_body, mesh=self.mesh,
                      in_specs=(PartitionSpec("core"),) * np_,
                      out_specs=(PartitionSpec("core"),) * len(out_names),
                      check_rep=False),
            keep_unused=True)

    def prep(self, in_maps):
        from jax.sharding import NamedSharding, PartitionSpec
        n = self.n
        cat = [np.concatenate([np.asarray(in_maps[c][k]) for c in range(n)], axis=0)
               for k in self.in_names]
        cat += [np.zeros((n * z.shape[0], *z.shape[1:]), z.dtype)
                for z in self.zero_outs]
        sh = NamedSharding(self.mesh, PartitionSpec("core"))
        return [jax.device_put(a, sh) for a in cat]

    def run(self, args):
        outs = self.fn(*args)
        jax.block_until_ready(outs)
        return outs

    def results(self, outs):
        n = self.n
        return [
            {name: np.asarray(outs[i]).reshape(n, *self.out_avals[i].shape)[c]
             for i, name in enumerate(self.out_names)}
            for c in range(n)
        ]


_EXEC = None


def _get_exec():
    global _EXEC
    if _EXEC is None:
        _EXEC = _Exec(build_program(), NC_)
    return _EXEC


def kernel(**inputs):
    e = _get_exec()
    in_maps = _host_prep(inputs)
    res = e.results(e.run(e.prep(in_maps)))
    out = np.zeros((B, D, L), np.float32)
    r = np.zeros((B, D, L), np.float32)
    for i in range(NC_):
        b = i // 4
        l0 = (i % 4) * TSL
        out[b][:, l0:l0 + TSL] = res[i]["o_out"]
        r[b][:, l0:l0 + TSL] = res[i]["r_out"]
    return out, r
